# revision 2
# baseline (speedup 1.0000x reference)
"""v6: v4 + proj chains inserted at chain boundaries inside the scores block
(fills PE stalls while ACT exp catches up; KV chain covers the exp tail
before the AV chain consumes pt).

Causal single-head attention (B=4, T=4096, E=1024, H=64) on 8 trn2 cores.

Sharding: core = b*2 + kh (batch x key-parity). exp(s) without max-sub;
host combines unnormalized numerator + row-sum (ones column of V).

Layout/schedule (v4):
- all matmul operands bf16; Q^T/K^T zero-padded to 128 contraction rows
  (HW runs contraction-64 matmuls at half rate)
- x^T fed as one [128, 4096] DMA per 512-column block (big DMAs amortize
  the ~0.5us fixed per-transfer cost)
- q-blocks processed in pairs (512-wide matmuls)
- PE accumulation chains (projection, AV) run CONTIGUOUSLY - interleaving
  other matmuls into a PSUM accumulation chain costs ~2x on HW.
  Per-pair PE order: [scores block] [proj block for next cb] [AV block].
  exp (ACT) pipelines behind the scores block; the proj block gives ACT
  time to drain before AV consumes the pt tiles.
"""
import sys
import numpy as np

sys.path.insert(0, "/opt/trn_rl_repo")

import concourse.bass as bass
import concourse.bacc as bacc
import concourse.tile as tile
from concourse import mybir
from concourse.bass_utils import run_bass_kernel_spmd

B, T, E, H = 4, 4096, 1024, 64
P = 128
QB = 256
NB = T // QB              # 16 local q-blocks
NPAIR = NB // 2           # 8 q-block pairs
NE = E // P               # 8 contraction chunks
F32 = mybir.dt.float32
BF16 = mybir.dt.bfloat16


def build_nc(loop_n=None, skip=()):
    nc = bacc.Bacc()
    xt_d = nc.declare_dram_parameter("xt", [P, NPAIR * NE * 512], BF16, isOutput=False)
    wq_d = nc.declare_dram_parameter("wq", [P, NE * H], BF16, isOutput=False)
    wkv_d = nc.declare_dram_parameter("wkv", [P, NE * 2 * H], BF16, isOutput=False)
    id_d = nc.declare_dram_parameter("ident", [H, H], BF16, isOutput=False)
    mask_d = nc.declare_dram_parameter("mask", [P, 2 * 2 * QB], BF16, isOutput=False)
    o_d = nc.declare_dram_parameter("o_un", [H + 1, T], F32, isOutput=True)

    with tile.TileContext(nc) as tc:
        with tc.tile_pool(name="const", bufs=1) as const, \
             tc.tile_pool(name="persist", bufs=1) as persist, \
             tc.tile_pool(name="xtp", bufs=3) as xtp, \
             tc.tile_pool(name="vttp", bufs=2) as vttp, \
             tc.tile_pool(name="ptp", bufs=9) as ptp, \
             tc.tile_pool(name="outp", bufs=3) as outp, \
             tc.tile_pool(name="acc", bufs=1, space="PSUM") as accp, \
             tc.tile_pool(name="stp", bufs=2, space="PSUM") as stp, \
             tc.tile_pool(name="otp", bufs=2, space="PSUM") as otp:

            wq_sb = const.tile([P, NE * H], BF16, tag="wq", name="wq_sb")
            nc.sync.dma_start(out=wq_sb, in_=wq_d[:, :])
            wkv_sb = const.tile([P, NE * 2 * H], BF16, tag="wkv", name="wkv_sb")
            nc.sync.dma_start(out=wkv_sb, in_=wkv_d[:, :])
            id_sb = const.tile([H, H], BF16, tag="ident", name="id_sb")
            nc.sync.dma_start(out=id_sb, in_=id_d[:, :])
            m_sb = const.tile([P, 2 * 2 * QB], BF16, tag="mask", name="m_sb")
            nc.sync.dma_start(out=m_sb, in_=mask_d[:, :])
            ones_sb = const.tile([P, 1], BF16, tag="ones", name="ones_sb")
            nc.vector.memset(ones_sb, 1.0)

            qt_tiles = [persist.tile([P, 2 * QB], BF16, tag=f"qt{i}", name=f"qt{i}")
                        for i in range(NPAIR)]
            kt_tiles = [persist.tile([P, QB], BF16, tag=f"kt{i}", name=f"kt{i}")
                        for i in range(NPAIR)]
            v_tiles = [persist.tile([P, H + 1], BF16, tag=f"v{i}", name=f"v{i}")
                       for i in range(2 * NPAIR)]
            for tl in qt_tiles + kt_tiles:
                nc.vector.memset(tl[H:P, :], 0.0)

            def body(_iv=None):
                def emit_dma(cb, box):
                    xt_t = xtp.tile([P, NE * 512], BF16, tag="xt", name="xt_t")
                    if "dma" not in skip:
                        nc.sync.dma_start(
                            out=xt_t, in_=xt_d[:, cb * NE * 512:(cb + 1) * NE * 512])
                    else:
                        nc.vector.memset(xt_t[:, 0:1], 0.0)
                    box.append(xt_t)

                def emit_q_chain(cb, box):
                    acc = accp.tile([P, 768], F32, tag="acc", name="acc")
                    xt_t = box[0]
                    if "proj" in skip:
                        nc.vector.memset(acc[:, 0:1], 0.0)
                        return acc
                    for e in range(NE):
                        nc.tensor.matmul(acc[0:H, 0:512],
                                         lhsT=wq_sb[:, e * H:(e + 1) * H],
                                         rhs=xt_t[:, e * 512:(e + 1) * 512],
                                         start=(e == 0), stop=(e == NE - 1))
                    return acc

                def emit_kv_chain(cb, box, acc):
                    if "proj" in skip:
                        return
                    xt_t = box[0]
                    for e in range(NE):
                        nc.tensor.matmul(acc[:, 512:768],
                                         lhsT=wkv_sb[:, e * P:(e + 1) * P],
                                         rhs=xt_t[:, e * 512:e * 512 + QB],
                                         start=(e == 0), stop=(e == NE - 1))

                def emit_proj_tail(cb, acc):
                    nc.vector.tensor_copy(qt_tiles[cb][0:H, :], acc[0:H, 0:512])
                    nc.vector.tensor_copy(kt_tiles[cb][0:H, :], acc[0:H, 512:768])
                    vtt = vttp.tile([H, QB], BF16, tag="vtt", name="vtt")
                    nc.vector.tensor_copy(vtt, acc[H:2 * H, 512:768])
                    for j in range(2):
                        i = 2 * cb + j
                        vtp = stp.tile([P, 1024], F32, tag="st",
                                       name="vtp").bitcast(BF16)[:, 0:H]
                        nc.tensor.transpose(vtp, vtt[:, j * P:(j + 1) * P], id_sb)
                        nc.vector.tensor_copy(v_tiles[i][:, 0:H], vtp)
                        nc.vector.tensor_copy(v_tiles[i][:, H:H + 1], ones_sb)

                # ---- software pipeline ----
                boxes = {0: []}
                emit_dma(0, boxes[0])
                acc0 = emit_q_chain(0, boxes[0])
                emit_kv_chain(0, boxes[0], acc0)
                emit_proj_tail(0, acc0)

                for c in range(NPAIR):
                    nkt = 2 * (c + 1)
                    nch = nkt // 2
                    if c + 1 < NPAIR:
                        boxes[c + 1] = []
                        emit_dma(c + 1, boxes[c + 1])

                    # scores block with proj chains inserted at chain
                    # boundaries (fill PE stalls while ACT catches up)
                    pt_tiles = []
                    acc = None

                    def schunk(ci):
                        st_ps = stp.tile([P, 1024], F32, tag="st", name="st_ps")
                        if "scores" in skip:
                            nc.vector.memset(st_ps[:, 0:1], 0.0)
                        else:
                            for j in range(2):
                                t = 2 * ci + j
                                nc.tensor.matmul(
                                    st_ps[:, j * 512:(j + 1) * 512],
                                    lhsT=kt_tiles[t // 2][:, (t % 2) * P:(t % 2 + 1) * P],
                                    rhs=qt_tiles[c], start=True, stop=True)
                        pt = ptp.tile([P, 1024], BF16, tag="pt", name="pt")
                        pt_tiles.append(pt)
                        if "exp" not in skip:
                            nc.scalar.activation(pt, st_ps,
                                                 mybir.ActivationFunctionType.Exp,
                                                 scale=0.125)
                        else:
                            nc.vector.memset(pt[:, 0:1], 0.0)
                        if ci == nch - 1 and "mask" not in skip:
                            nc.gpsimd.tensor_mul(pt, pt, m_sb)

                    have_proj = c + 1 < NPAIR
                    q_at = min(2, nch - 1) if have_proj else None
                    for ci in range(nch):
                        schunk(ci)
                        if have_proj and ci == q_at:
                            acc = emit_q_chain(c + 1, boxes[c + 1])
                    if have_proj:
                        emit_kv_chain(c + 1, boxes[c + 1], acc)

                    # AV block (contiguous accumulation chain)
                    ot_f = otp.tile([P, 512], F32, tag="ot", name="ot")
                    ot = ot_f[0:H + 1, :]
                    if "av" in skip:
                        nc.vector.memset(ot_f[:, 0:1], 0.0)
                    else:
                        for ci in range(nch):
                            for j in range(2):
                                t = 2 * ci + j
                                nc.tensor.matmul(
                                    ot, lhsT=v_tiles[t],
                                    rhs=pt_tiles[ci][:, j * 512:(j + 1) * 512],
                                    start=(t == 0), stop=(t == nkt - 1))

                    if acc is not None:
                        emit_proj_tail(c + 1, acc)

                    o_t = outp.tile([H + 1, 512], F32, tag="o", name="o_t")
                    nc.vector.tensor_copy(o_t, ot)
                    nc.sync.dma_start(out=o_d[:, c * 512:(c + 1) * 512],
                                      in_=o_t[:, :])

            if loop_n is not None and loop_n > 1:
                with tc.For_i(0, loop_n, 1):
                    body()
            else:
                body()

    nc.compile()
    return nc


def _host_prep(x, Wq, Wk, Wv):
    import ml_dtypes
    bf16 = ml_dtypes.bfloat16

    x = np.asarray(x, np.float32)
    xt_all = np.ascontiguousarray(x.transpose(0, 2, 1)).astype(bf16)  # [B, E, T]
    wq_r = np.ascontiguousarray(
        np.asarray(Wq, np.float32).reshape(NE, P, H).transpose(1, 0, 2)
    ).reshape(P, NE * H).astype(bf16)
    wkv = np.concatenate([np.asarray(Wk, np.float32).reshape(NE, P, H),
                          np.asarray(Wv, np.float32).reshape(NE, P, H)], axis=2)
    wkv_r = np.ascontiguousarray(wkv.transpose(1, 0, 2)).reshape(
        P, NE * 2 * H).astype(bf16)
    ident = np.eye(H, dtype=bf16)

    kq = np.arange(P)[:, None]
    qq = np.arange(QB)[None, :]
    masks = []
    for kh in range(2):
        m = np.zeros((P, 2, 2 * QB), np.float32)
        for j in range(2):
            m[:, j, 0:QB] = ((j * P + kq) <= qq)
            m[:, j, QB:2 * QB] = 1.0 if kh == 0 else 0.0
        masks.append(m.reshape(P, 2 * 2 * QB).astype(bf16))

    in_maps = []
    for b in range(B):
        for kh in range(2):
            xt = xt_all[b]
            if kh == 1:
                xt = xt.reshape(E, NB // 2, 2, QB)[:, :, ::-1, :].reshape(E, T)
            xp = np.ascontiguousarray(
                xt.reshape(NE, P, NPAIR, 512).transpose(1, 2, 0, 3)
            ).reshape(P, NPAIR * NE * 512)
            in_maps.append({"xt": xp, "wq": wq_r, "wkv": wkv_r,
                            "ident": ident, "mask": masks[kh]})
    return in_maps


def _host_combine(results):
    out = np.zeros((B, T, H), np.float32)
    for b in range(B):
        o0 = results[2 * b]["o_un"]
        o1 = results[2 * b + 1]["o_un"]
        o1 = o1.reshape(H + 1, NB // 2, 2, QB)[:, :, ::-1, :].reshape(H + 1, T)
        s = o0 + o1
        out[b] = (s[:H] / s[H:H + 1]).T
    return out


_NC_CACHE = {}


def kernel(x, Wq, Wk, Wv):
    key = "main"
    if key not in _NC_CACHE:
        _NC_CACHE[key] = build_nc()
    nc = _NC_CACHE[key]
    in_maps = _host_prep(x, Wq, Wk, Wv)
    res = run_bass_kernel_spmd(nc, in_maps, core_ids=list(range(8)))
    return _host_combine(res.results)
